# revision 7
# baseline (speedup 1.0000x reference)
"""nn_MultiHeadAttention kernel for 8 Trainium2 NeuronCores.

Sharding: 8 cores = 4 batches (data parallel) x 2 head-groups of 8 heads
(tensor parallel). Each core computes its batch's QKV projection for its
head group (column-parallel), RoPE, causal attention, and a partial
out-projection (row-parallel). Host sums the two partials per batch and
adds the output bias.

v2 changes vs v1 (all aimed at tensor-engine occupancy):
  - Z row-sums moved off PE: exp tiles are accumulated on the Pool engine
    (gpsimd) into zacc, then ONE ones[128x128] matmul per 512-chunk both
    reduces over partitions and broadcasts Z to all 128 partitions
    (replaces 40 Z-matmuls + GPSIMD partition_broadcast per head).
  - v bias via DVE add on PSUM eviction (drops 64 K=1 matmuls).
  - bf16 for q/k/v spills, exp tiles, attn output, Wo (same PE speed,
    half the DMA bytes and SBUF footprint).
  - bufs=2 on all per-head pools: head h+1 DMA + RoPE overlap head h PE.
  - phase 1 reordered (q0,k0 first, v interleaved) so phase 2 prefetch
    starts early; chunk finalization emitted late to avoid PE stalls.
"""

import sys

if "/opt/trn_rl_repo" not in sys.path:
    sys.path.insert(0, "/opt/trn_rl_repo")

import numpy as np
import ml_dtypes

import concourse.bass as bass
import concourse.bacc as bacc
import concourse.mybir as mybir
import concourse.tile as tile
from concourse.bass_utils import run_bass_kernel_spmd

F32 = mybir.dt.float32
F32R = mybir.dt.float32r
BF16 = mybir.dt.bfloat16

B, T, C = 4, 2048, 2048
H = 16            # total heads
HG = 8            # heads per core (group)
D = 128           # head dim
GC = HG * D       # channels per group = 1024
SCALE = 1.0 / float(np.sqrt(D))
MASKVAL = -30000.0
N_CORES = 8

KT = C // 128     # 16 K tiles
TT = T // 128     # 16 T tiles
TC = T // 512     # 4 T chunks of 512


def build_program(iters=1):
    nc = bacc.Bacc("TRN2", target_bir_lowering=False, debug=False)

    xT = nc.dram_tensor("xT", [C, T], F32R, kind="ExternalInput").ap()
    wq = nc.dram_tensor("wq", [C, GC], F32R, kind="ExternalInput").ap()
    wk = nc.dram_tensor("wk", [C, GC], F32R, kind="ExternalInput").ap()
    wv = nc.dram_tensor("wv", [C, GC], F32R, kind="ExternalInput").ap()
    bq = nc.dram_tensor("bq", [GC, 1], F32, kind="ExternalInput").ap()
    bk = nc.dram_tensor("bk", [GC, 1], F32, kind="ExternalInput").ap()
    bvb = nc.dram_tensor("bvb", [128, GC], F32, kind="ExternalInput").ap()
    wo = nc.dram_tensor("wo", [GC, C], BF16, kind="ExternalInput").ap()
    sin2 = nc.dram_tensor("sin2", [128, T], BF16, kind="ExternalInput").ap()
    cos2 = nc.dram_tensor("cos2", [128, T], BF16, kind="ExternalInput").ap()
    masks = nc.dram_tensor("masks", [4, 128, 512], BF16, kind="ExternalInput").ap()
    ident = nc.dram_tensor("ident", [128, 128], BF16, kind="ExternalInput").ap()
    ones_sq = nc.dram_tensor("ones_sq", [128, 128], F32R, kind="ExternalInput").ap()
    y = nc.dram_tensor("y", [T, C], F32, kind="ExternalOutput").ap()

    with tile.TileContext(nc) as tc:
        with tc.tile_pool(name="dram", bufs=1, space="DRAM") as dpool, \
             tc.tile_pool(name="consts", bufs=1) as rpool:
            # DRAM scratch: roped... raw q,k rows and v, all bf16
            qk_d = [dpool.tile([128, T], BF16, tag=f"qkd{m}", name=f"qkd{m}")
                    for m in range(16)]
            # per-head V: block t at cols [t*128,(t+1)*128), partition = row
            # within tile t (the PV lhsT layout) -> one contiguous load/head
            vh_d = [dpool.tile([128, TT * 128], BF16, tag=f"vhd{h}", name=f"vhd{h}")
                    for h in range(HG)]
            masks_sb = rpool.tile([128, 4 * 512], BF16, tag="masks")
            ident_sb = rpool.tile([128, 128], BF16, tag="ident")
            ones_sq_sb = rpool.tile([128, 128], F32R, tag="ones_sq")
            bvb_sb = rpool.tile([128, GC], F32, tag="bvb")

            def full_body(iv):
                nc.sync.dma_start(
                    out=masks_sb[:].rearrange("p (r c) -> p r c", r=4),
                    in_=masks.rearrange("r p c -> p r c"),
                )
                nc.sync.dma_start(out=ident_sb[:], in_=ident)
                nc.sync.dma_start(out=ones_sq_sb[:], in_=ones_sq)
                nc.sync.dma_start(out=bvb_sb[:], in_=bvb)

                # ---------------- Phase 1: QKV projection ----------------
                with tc.tile_pool(name="p1x", bufs=1) as xpool, \
                     tc.tile_pool(name="p1w", bufs=2) as wpool, \
                     tc.tile_pool(name="p1wv", bufs=2) as wvpool, \
                     tc.tile_pool(name="p1t", bufs=3) as tpool, \
                     tc.tile_pool(name="p1ps", bufs=2, space="PSUM") as pspool:
                    xt_sb = []
                    for k in range(KT):
                        t = xpool.tile([128, T], F32R, tag=f"xt{k}", name=f"xt{k}")
                        nc.sync.dma_start(out=t[:], in_=xT[k * 128:(k + 1) * 128, :])
                        xt_sb.append(t)

                    def qk_row(m):
                        # one 128-channel output row of q (m<8) or k (m>=8),
                        # evicted bf16 in [D, T] layout
                        w = wq if m < 8 else wk
                        bias = bq if m < 8 else bk
                        row = m % 8
                        wrow = wpool.tile([128, KT * 128], F32R, tag="wrow")
                        nc.sync.dma_start(
                            out=wrow[:].rearrange("p (k c) -> p k c", k=KT),
                            in_=w[:, row * 128:(row + 1) * 128].rearrange(
                                "(k p) c -> p k c", p=128
                            ),
                        )
                        bias_t = wpool.tile([128, 1], F32, tag="bias")
                        nc.sync.dma_start(
                            out=bias_t[:], in_=bias[row * 128:(row + 1) * 128, :]
                        )
                        for n in range(TC):
                            ps = pspool.tile([128, 512], F32, tag="pqk")
                            for k in range(KT):
                                nc.tensor.matmul(
                                    ps[:],
                                    wrow[:, k * 128:(k + 1) * 128],
                                    xt_sb[k][:, n * 512:(n + 1) * 512],
                                    start=(k == 0),
                                    stop=(k == KT - 1),
                                )
                            qt = tpool.tile([128, 512], BF16, tag="qt")
                            nc.vector.tensor_scalar_add(qt[:], ps[:], bias_t[:])
                            nc.sync.dma_start(
                                out=qk_d[m][:, n * 512:(n + 1) * 512], in_=qt[:]
                            )

                    def v_nd(nd):
                        # 256 v channels (2 heads), [T, D] layout, bias on DVE
                        ndsl = slice(nd * 256, (nd + 1) * 256)
                        wvc = wvpool.tile([128, KT * 256], F32R, tag="wvc")
                        nc.sync.dma_start(
                            out=wvc[:].rearrange("p (k c) -> p k c", k=KT),
                            in_=wv[:, ndsl].rearrange("(k p) c -> p k c", p=128),
                        )
                        for t in range(TT):
                            ps = pspool.tile([128, 256], F32, tag="pv")
                            for k in range(KT):
                                nc.tensor.matmul(
                                    ps[:],
                                    xt_sb[k][:, t * 128:(t + 1) * 128],
                                    wvc[:, k * 256:(k + 1) * 256],
                                    start=(k == 0),
                                    stop=(k == KT - 1),
                                )
                            vt = tpool.tile([128, 256], BF16, tag="vt")
                            nc.vector.tensor_add(vt[:], ps[:], bvb_sb[:, ndsl])
                            tsl = slice(t * 128, (t + 1) * 128)
                            nc.sync.dma_start(
                                out=vh_d[2 * nd][:, tsl], in_=vt[:, 0:128]
                            )
                            nc.sync.dma_start(
                                out=vh_d[2 * nd + 1][:, tsl], in_=vt[:, 128:256]
                            )

                    # order: head 0/1 q,k rows and their v first, so phase 2
                    # prefetch can begin while phase 1 is still computing
                    for g in range(4):
                        qk_row(2 * g)
                        qk_row(8 + 2 * g)
                        qk_row(2 * g + 1)
                        qk_row(8 + 2 * g + 1)
                        v_nd(g)

                # ---------------- Phases 2+3 ----------------
                with tc.tile_pool(name="p23attn", bufs=1) as apool:
                    attn_sb = [
                        apool.tile([128, T], BF16, tag=f"attn{h}", name=f"attn{h}")
                        for h in range(HG)
                    ]
                    sin_sb = apool.tile([128, T], BF16, tag="sin")
                    cos_sb = apool.tile([128, T], BF16, tag="cos")
                    nc.sync.dma_start(out=sin_sb[:], in_=sin2)
                    nc.sync.dma_start(out=cos_sb[:], in_=cos2)

                    # ---- Phase 2: attention per head ----
                    with tc.tile_pool(name="p2raw", bufs=2) as rawpool, \
                         tc.tile_pool(name="p2rope", bufs=2) as ropepool, \
                         tc.tile_pool(name="p2e", bufs=4) as epool, \
                         tc.tile_pool(name="p2z", bufs=2) as zpool, \
                         tc.tile_pool(name="p2n", bufs=2) as npool, \
                         tc.tile_pool(name="p2ps", bufs=2, space="PSUM") as ps2, \
                         tc.tile_pool(name="p2po", bufs=2, space="PSUM") as po2:
                        for h in range(HG):
                            qraw = rawpool.tile([128, T], BF16, tag="qraw")
                            kraw = rawpool.tile([128, T], BF16, tag="kraw")
                            qsw = rawpool.tile([128, T], BF16, tag="qsw")
                            ksw = rawpool.tile([128, T], BF16, tag="ksw")
                            vh = rawpool.tile([128, TT * 128], BF16, tag="vh")
                            nc.sync.dma_start(out=qraw[:], in_=qk_d[h][:])
                            nc.sync.dma_start(out=kraw[:], in_=qk_d[8 + h][:])
                            # partition-swapped copies (halves exchanged)
                            nc.sync.dma_start(out=qsw[0:64, :], in_=qk_d[h][64:128, :])
                            nc.sync.dma_start(out=qsw[64:128, :], in_=qk_d[h][0:64, :])
                            nc.sync.dma_start(
                                out=ksw[0:64, :], in_=qk_d[8 + h][64:128, :]
                            )
                            nc.sync.dma_start(
                                out=ksw[64:128, :], in_=qk_d[8 + h][0:64, :]
                            )
                            nc.sync.dma_start(out=vh[:], in_=vh_d[h][:])
                            # RoPE: ro = raw*cos2 + swapped(raw)*[-sin; +sin]
                            qr = ropepool.tile([128, T], F32R, tag="qr")
                            kr = ropepool.tile([128, T], F32R, tag="kr")
                            for raw, sw, ro in ((qraw, qsw, qr), (kraw, ksw, kr)):
                                tmp = ropepool.tile([128, T], F32R, tag="tmp")
                                nc.vector.tensor_mul(ro[:], raw[:], cos_sb[:])
                                nc.vector.tensor_mul(tmp[:], sw[:], sin_sb[:])
                                nc.vector.tensor_add(ro[:], ro[:], tmp[:])

                            # flat pair pipeline: PV for pair p is emitted
                            # while pair p+1's scores run, so the PE never
                            # sits behind the Act-engine exp in its queue.
                            # finalize(n) is emitted two pairs into chunk n+1.
                            pairs = [(n, jp) for n in range(TC)
                                     for jp in range(2 * (n + 1))]
                            chunk_state = {}
                            prev = None     # (n, jp, pexp) awaiting PV
                            fin = None      # pending finalize closure

                            def emit_pv(p):
                                n, jp, pexp = p
                                jmax = 4 * (n + 1)
                                ps_o = chunk_state[n][0]
                                for u in range(2):
                                    j = 2 * jp + u
                                    half = slice(u * 512, (u + 1) * 512)
                                    nc.tensor.matmul(
                                        ps_o[:],
                                        vh[:, j * 128:(j + 1) * 128],
                                        pexp[:, half],
                                        start=(j == 0),
                                        stop=(j == jmax - 1),
                                    )

                            def make_finalize(n, h=h):
                                ps_o, zacc = chunk_state[n]
                                qsl = slice(n * 512, (n + 1) * 512)

                                def finalize():
                                    # one matmul: Z = colsum(zacc) broadcast
                                    # to all 128 partitions
                                    ps_z = po2.tile([128, 512], F32, tag="pz")
                                    nc.tensor.matmul(
                                        ps_z[:], ones_sq_sb[:], zacc[:],
                                        start=True, stop=True,
                                    )
                                    rzb = npool.tile([128, 512], F32, tag="rzb")
                                    nc.vector.reciprocal(rzb[:], ps_z[:])
                                    nc.vector.tensor_mul(
                                        attn_sb[h][:, qsl], ps_o[:], rzb[:]
                                    )
                                return finalize

                            for n, jp in pairs:
                                if jp == 0:
                                    ps_o = po2.tile(
                                        [128, 512], F32, tag="po", name="ps_o"
                                    )
                                    zacc0 = zpool.tile(
                                        [128, 512], F32R, tag="zacc", name="zacc"
                                    )
                                    chunk_state[n] = (ps_o, zacc0)
                                zacc = chunk_state[n][1]
                                qsl = slice(n * 512, (n + 1) * 512)
                                ps_s = ps2.tile([128, 1024], F32, tag="ps")
                                for u in range(2):
                                    j = 2 * jp + u
                                    half = slice(u * 512, (u + 1) * 512)
                                    diag = (j // 4) == n
                                    nc.tensor.matmul(
                                        ps_s[:, half],
                                        kr[:, j * 128:(j + 1) * 128],
                                        qr[:, qsl],
                                        start=True,
                                        stop=not diag,
                                    )
                                    if diag:
                                        r = j % 4
                                        nc.tensor.matmul(
                                            ps_s[:, half],
                                            ident_sb[:],
                                            masks_sb[:, r * 512:(r + 1) * 512],
                                            start=False,
                                            stop=True,
                                        )
                                pexp = epool.tile([128, 1024], BF16, tag="pexp")
                                nc.scalar.activation(
                                    pexp[:],
                                    ps_s[:],
                                    mybir.ActivationFunctionType.Exp,
                                    scale=SCALE,
                                )
                                # Z partial sums on the Pool engine
                                if jp == 0:
                                    nc.gpsimd.tensor_add(
                                        zacc[:], pexp[:, 0:512], pexp[:, 512:1024]
                                    )
                                else:
                                    nc.gpsimd.tensor_add(
                                        zacc[:], zacc[:], pexp[:, 0:512]
                                    )
                                    nc.gpsimd.tensor_add(
                                        zacc[:], zacc[:], pexp[:, 512:1024]
                                    )
                                if prev is not None:
                                    emit_pv(prev)
                                if fin is not None and jp == 1:
                                    fin()
                                    fin = None
                                prev = (n, jp, pexp)
                                if jp == 2 * (n + 1) - 1:
                                    fin = make_finalize(n)
                            emit_pv(prev)
                            fin()

                    # ---- Phase 3: out projection ----
                    with tc.tile_pool(name="p3w", bufs=2) as wpool3, \
                         tc.tile_pool(name="p3t", bufs=3) as tpool3, \
                         tc.tile_pool(name="p3ps", bufs=2, space="PSUM") as ps3:
                        for n in range(4):
                            woc = wpool3.tile([128, HG * 512], BF16, tag="woc")
                            nc.sync.dma_start(
                                out=woc[:].rearrange("p (h c) -> p h c", h=HG),
                                in_=wo[:, n * 512:(n + 1) * 512].rearrange(
                                    "(h p) c -> p h c", p=128
                                ),
                            )
                            for m in range(TT):
                                ps_y = ps3.tile([128, 512], F32, tag="py")
                                for h in range(HG):
                                    nc.tensor.matmul(
                                        ps_y[:],
                                        attn_sb[h][:, m * 128:(m + 1) * 128],
                                        woc[:, h * 512:(h + 1) * 512],
                                        start=(h == 0),
                                        stop=(h == HG - 1),
                                    )
                                yt = tpool3.tile([128, 512], F32, tag="yt")
                                nc.scalar.copy(yt[:], ps_y[:])
                                nc.sync.dma_start(
                                    out=y[m * 128:(m + 1) * 128,
                                          n * 512:(n + 1) * 512],
                                    in_=yt[:],
                                )

            if iters == 1:
                full_body(None)
            else:
                with tc.For_i(0, iters, 1) as iv:
                    full_body(iv)

    nc.compile()
    return nc


def make_host_inputs(x, Wqkv, bqkv, Wo):
    """Per-core input maps (host-side sharding)."""
    half = D // 2
    freq = np.arange(half, dtype=np.float64)
    theta = 1.0 / (10000.0 ** (2.0 * freq / D))
    pos = np.arange(T, dtype=np.float64)
    ang = pos[:, None] * theta[None, :]          # [T, half]
    sinT = np.sin(ang).T.astype(np.float32)      # [half, T]
    cosT = np.cos(ang).T.astype(np.float32)
    # sign folded into the sin table for the partition-swap RoPE form
    sin2 = np.concatenate([-sinT, sinT], axis=0).astype(ml_dtypes.bfloat16)
    cos2 = np.concatenate([cosT, cosT], axis=0).astype(ml_dtypes.bfloat16)

    masks = np.zeros((4, 128, 512), dtype=np.float32)
    f = np.arange(512)[None, :]
    p = np.arange(128)[:, None]
    for r in range(4):
        masks[r] = np.where(f >= r * 128 + p, 0.0, MASKVAL)
    masks = masks.astype(ml_dtypes.bfloat16)
    ident = np.eye(128, dtype=np.float32).astype(ml_dtypes.bfloat16)
    ones_sq = np.ones((128, 128), dtype=np.float32)

    xT = [np.ascontiguousarray(x[b].T) for b in range(B)]
    in_maps = []
    for core in range(N_CORES):
        b, g = core // 2, core % 2
        cs = slice(g * GC, (g + 1) * GC)
        in_maps.append({
            "xT": xT[b],
            "wq": np.ascontiguousarray(Wqkv[:, :C][:, cs]),
            "wk": np.ascontiguousarray(Wqkv[:, C:2 * C][:, cs]),
            "wv": np.ascontiguousarray(Wqkv[:, 2 * C:][:, cs]),
            "bq": np.ascontiguousarray(bqkv[:C][cs].reshape(GC, 1)),
            "bk": np.ascontiguousarray(bqkv[C:2 * C][cs].reshape(GC, 1)),
            "bvb": np.ascontiguousarray(
                np.tile(bqkv[2 * C:][cs].reshape(1, GC), (128, 1))
            ),
            "wo": np.ascontiguousarray(Wo[cs, :]).astype(ml_dtypes.bfloat16),
            "sin2": sin2,
            "cos2": cos2,
            "masks": masks,
            "ident": ident,
            "ones_sq": ones_sq,
        })
    return in_maps


_PROGRAM_CACHE = {}


def get_program(iters=1):
    if iters not in _PROGRAM_CACHE:
        _PROGRAM_CACHE[iters] = build_program(iters)
    return _PROGRAM_CACHE[iters]


def kernel(x, Wqkv, bqkv, Wo, bo):
    x = np.asarray(x, dtype=np.float32)
    Wqkv = np.asarray(Wqkv, dtype=np.float32)
    bqkv = np.asarray(bqkv, dtype=np.float32)
    Wo = np.asarray(Wo, dtype=np.float32)
    bo = np.asarray(bo, dtype=np.float32)

    nc = get_program(1)
    in_maps = make_host_inputs(x, Wqkv, bqkv, Wo)
    res = run_bass_kernel_spmd(nc, in_maps, list(range(N_CORES)))

    out = np.empty((B, T, C), dtype=np.float32)
    for b in range(B):
        out[b] = res.results[2 * b]["y"] + res.results[2 * b + 1]["y"] + bo
    return out


# revision 10
# speedup vs baseline: 3.8387x; 3.8387x over previous
"""nn_MultiHeadAttention kernel for 8 Trainium2 NeuronCores.

Sharding: 8 cores = 4 batches (data parallel) x 2 head-groups of 8 heads
(tensor parallel). Each core computes its batch's QKV projection for its
head group (column-parallel), RoPE, causal attention, and a partial
out-projection (row-parallel). Host sums the two partials per batch and
adds the output bias.

v4 design notes (engine budget per head, targeting ~20 us/head phase 2):
  - Phase 1 evictions fuse bias + RoPE tables: spill q*cos and q*sin
    (sin table pre-swapped) via one scalar_tensor_tensor each, so phase-2
    RoPE is a single all-bf16 DVE add per q/k (2x DVE mode).
  - Causal mask: binary bf16 multiply on pexp (DVE 2x), no mask matmuls.
  - Softmax Z: chunks 0,1 via [1,512] ones-column matmuls on PE; chunks
    2,3 via bf16 adds on DVE; balances PE~20 / DVE~20 / Act~18 us/head.
    (Pool/GPSIMD runs elementwise at 0.42 efficiency - only used for the
    two cheap partition broadcasts per head.)
  - Flat pair pipeline: PV + Z matmuls for pair p emitted while pair
    p+1's scores run, so the PE never waits on the Act-engine exp.
  - bf16 everywhere off the QKV/score matmul accumulation paths.
  - Spill writes go on the Act DMA queue, loads on the SP queue, so
    phase-2 head-0 loads dispatch mid-phase-1.
"""

import sys

if "/opt/trn_rl_repo" not in sys.path:
    sys.path.insert(0, "/opt/trn_rl_repo")

import numpy as np
import ml_dtypes

import concourse.bass as bass
import concourse.bacc as bacc
import concourse.mybir as mybir
import concourse.tile as tile
from concourse.bass_utils import run_bass_kernel_spmd

F32 = mybir.dt.float32
F32R = mybir.dt.float32r
BF16 = mybir.dt.bfloat16
ADD = mybir.AluOpType.add
MULT = mybir.AluOpType.mult

B, T, C = 4, 2048, 2048
H = 16            # total heads
HG = 8            # heads per core (group)
D = 128           # head dim
GC = HG * D       # channels per group = 1024
SCALE = 1.0 / float(np.sqrt(D))
N_CORES = 8

KT = C // 128     # 16 K tiles
TT = T // 128     # 16 T tiles
TC = T // 512     # 4 T chunks of 512

PE_Z_CHUNKS = 2   # chunks 0..1 do Z on PE, the rest on DVE


def build_program(iters=1):
    nc = bacc.Bacc("TRN2", target_bir_lowering=False, debug=False)

    xT = nc.dram_tensor("xT", [C, T], F32R, kind="ExternalInput").ap()
    wq = nc.dram_tensor("wq", [C, GC], F32R, kind="ExternalInput").ap()
    wk = nc.dram_tensor("wk", [C, GC], F32R, kind="ExternalInput").ap()
    wv = nc.dram_tensor("wv", [C, GC], F32R, kind="ExternalInput").ap()
    bq = nc.dram_tensor("bq", [GC, 1], F32, kind="ExternalInput").ap()
    bk = nc.dram_tensor("bk", [GC, 1], F32, kind="ExternalInput").ap()
    bvb = nc.dram_tensor("bvb", [128, GC], F32, kind="ExternalInput").ap()
    wo = nc.dram_tensor("wo", [GC, C], BF16, kind="ExternalInput").ap()
    sin2p = nc.dram_tensor("sin2p", [128, T], BF16, kind="ExternalInput").ap()
    cos2 = nc.dram_tensor("cos2", [128, T], BF16, kind="ExternalInput").ap()
    masks2 = nc.dram_tensor("masks2", [2, 128, 1024], BF16, kind="ExternalInput").ap()
    ones_bf = nc.dram_tensor("ones_bf", [128, 128], BF16, kind="ExternalInput").ap()
    onescol_bf = nc.dram_tensor("onescol_bf", [128, 1], BF16, kind="ExternalInput").ap()
    y = nc.dram_tensor("y", [T, C], F32, kind="ExternalOutput").ap()

    with tile.TileContext(nc) as tc:
        with tc.tile_pool(name="dram", bufs=1, space="DRAM") as dpool, \
             tc.tile_pool(name="consts", bufs=1) as rpool:
            # DRAM scratch (bf16): q*cos rows (m=0..7 q, 8..15 k), q*sin rows,
            # per-head V in PV-lhsT layout
            qc_d = [dpool.tile([128, T], BF16, tag=f"qcd{m}", name=f"qcd{m}")
                    for m in range(16)]
            qs_d = [dpool.tile([128, T], BF16, tag=f"qsd{m}", name=f"qsd{m}")
                    for m in range(16)]
            vh_d = [dpool.tile([128, TT * 128], BF16, tag=f"vhd{h}", name=f"vhd{h}")
                    for h in range(HG)]
            masks_sb = rpool.tile([128, 2 * 1024], BF16, tag="masks")
            ones_sb = rpool.tile([128, 128], BF16, tag="ones")
            onescol_sb = rpool.tile([128, 1], BF16, tag="onescol")
            bvb_sb = rpool.tile([128, GC], F32, tag="bvb")
            sin_sb = rpool.tile([128, T], BF16, tag="sin")
            cos_sb = rpool.tile([128, T], BF16, tag="cos")

            def full_body(iv):
                nc.sync.dma_start(
                    out=masks_sb[:].rearrange("p (r c) -> p r c", r=2),
                    in_=masks2.rearrange("r p c -> p r c"),
                )
                nc.sync.dma_start(out=ones_sb[:], in_=ones_bf)
                nc.sync.dma_start(out=onescol_sb[:], in_=onescol_bf)
                nc.sync.dma_start(out=bvb_sb[:], in_=bvb)
                nc.sync.dma_start(out=sin_sb[:], in_=sin2p)
                nc.sync.dma_start(out=cos_sb[:], in_=cos2)

                # ---------------- Phase 1: QKV projection ----------------
                with tc.tile_pool(name="p1x", bufs=1) as xpool, \
                     tc.tile_pool(name="p1w", bufs=2) as wpool, \
                     tc.tile_pool(name="p1wv", bufs=2) as wvpool, \
                     tc.tile_pool(name="p1t", bufs=3) as tpool, \
                     tc.tile_pool(name="p1ps", bufs=2, space="PSUM") as pspool:
                    xt_sb = []
                    for k in range(KT):
                        t = xpool.tile([128, T], F32R, tag=f"xt{k}", name=f"xt{k}")
                        nc.sync.dma_start(out=t[:], in_=xT[k * 128:(k + 1) * 128, :])
                        xt_sb.append(t)

                    def qk_row(m):
                        # one 128-channel row of q (m<8) or k (m>=8); evict
                        # (x@W + b)*cos and (x@W + b)*sin_swapped, bf16 [D,T]
                        w = wq if m < 8 else wk
                        bias = bq if m < 8 else bk
                        row = m % 8
                        wrow = wpool.tile([128, KT * 128], F32R, tag="wrow")
                        nc.sync.dma_start(
                            out=wrow[:].rearrange("p (k c) -> p k c", k=KT),
                            in_=w[:, row * 128:(row + 1) * 128].rearrange(
                                "(k p) c -> p k c", p=128
                            ),
                        )
                        bias_t = wpool.tile([128, 1], F32, tag="bias")
                        nc.sync.dma_start(
                            out=bias_t[:], in_=bias[row * 128:(row + 1) * 128, :]
                        )
                        for n in range(TC):
                            csl = slice(n * 512, (n + 1) * 512)
                            ps = pspool.tile([128, 512], F32, tag="pqk")
                            for k in range(KT):
                                nc.tensor.matmul(
                                    ps[:],
                                    wrow[:, k * 128:(k + 1) * 128],
                                    xt_sb[k][:, csl],
                                    start=(k == 0),
                                    stop=(k == KT - 1),
                                )
                            qc = tpool.tile([128, 512], BF16, tag="qc")
                            nc.vector.scalar_tensor_tensor(
                                qc[:], ps[:], bias_t[:], cos_sb[:, csl], ADD, MULT
                            )
                            nc.scalar.dma_start(out=qc_d[m][:, csl], in_=qc[:])
                            qs = tpool.tile([128, 512], BF16, tag="qs")
                            nc.vector.scalar_tensor_tensor(
                                qs[:], ps[:], bias_t[:], sin_sb[:, csl], ADD, MULT
                            )
                            nc.scalar.dma_start(out=qs_d[m][:, csl], in_=qs[:])

                    def v_nd(nd):
                        # 256 v channels (2 heads), [T, D] layout, bias on DVE
                        ndsl = slice(nd * 256, (nd + 1) * 256)
                        wvc = wvpool.tile([128, KT * 256], F32R, tag="wvc")
                        nc.sync.dma_start(
                            out=wvc[:].rearrange("p (k c) -> p k c", k=KT),
                            in_=wv[:, ndsl].rearrange("(k p) c -> p k c", p=128),
                        )
                        for t in range(TT):
                            ps = pspool.tile([128, 256], F32, tag="pv")
                            for k in range(KT):
                                nc.tensor.matmul(
                                    ps[:],
                                    xt_sb[k][:, t * 128:(t + 1) * 128],
                                    wvc[:, k * 256:(k + 1) * 256],
                                    start=(k == 0),
                                    stop=(k == KT - 1),
                                )
                            vt = tpool.tile([128, 256], BF16, tag="vt")
                            nc.vector.tensor_add(vt[:], ps[:], bvb_sb[:, ndsl])
                            tsl = slice(t * 128, (t + 1) * 128)
                            nc.scalar.dma_start(
                                out=vh_d[2 * nd][:, tsl], in_=vt[:, 0:128]
                            )
                            nc.scalar.dma_start(
                                out=vh_d[2 * nd + 1][:, tsl], in_=vt[:, 128:256]
                            )

                    # head 0/1's rows and v first -> early phase-2 prefetch
                    for g in range(4):
                        qk_row(2 * g)
                        qk_row(8 + 2 * g)
                        qk_row(2 * g + 1)
                        qk_row(8 + 2 * g + 1)
                        v_nd(g)

                # ---------------- Phases 2+3 ----------------
                with tc.tile_pool(name="p23attn", bufs=1) as apool, \
                     tc.tile_pool(name="p2raw", bufs=2) as rawpool, \
                     tc.tile_pool(name="p2rope", bufs=2) as ropepool, \
                     tc.tile_pool(name="p2e", bufs=4) as epool, \
                     tc.tile_pool(name="p2z", bufs=2) as zpool, \
                     tc.tile_pool(name="p2n", bufs=2) as npool:
                    attn_sb = [
                        apool.tile([128, T], BF16, tag=f"attn{h}", name=f"attn{h}")
                        for h in range(HG)
                    ]

                    def emit_loads(h):
                        qc_t = rawpool.tile([128, T], BF16, tag="qc", name="qc_t")
                        kc_t = rawpool.tile([128, T], BF16, tag="kc", name="kc_t")
                        qs_t = rawpool.tile([128, T], BF16, tag="qsw", name="qs_t")
                        ks_t = rawpool.tile([128, T], BF16, tag="ksw", name="ks_t")
                        vh = rawpool.tile([128, TT * 128], BF16, tag="vh", name="vh")
                        nc.sync.dma_start(out=qc_t[:], in_=qc_d[h][:])
                        nc.sync.dma_start(out=kc_t[:], in_=qc_d[8 + h][:])
                        # partition-swapped loads of the sin spills
                        nc.sync.dma_start(out=qs_t[0:64, :], in_=qs_d[h][64:128, :])
                        nc.sync.dma_start(out=qs_t[64:128, :], in_=qs_d[h][0:64, :])
                        nc.sync.dma_start(
                            out=ks_t[0:64, :], in_=qs_d[8 + h][64:128, :]
                        )
                        nc.sync.dma_start(
                            out=ks_t[64:128, :], in_=qs_d[8 + h][0:64, :]
                        )
                        nc.sync.dma_start(out=vh[:], in_=vh_d[h][:])
                        return qc_t, kc_t, qs_t, ks_t, vh

                    def emit_rope(loaded):
                        qc_t, kc_t, qs_t, ks_t, vh = loaded
                        qr = ropepool.tile([128, T], BF16, tag="qr", name="qr")
                        kr = ropepool.tile([128, T], BF16, tag="kr", name="kr")
                        nc.vector.tensor_add(qr[:], qc_t[:], qs_t[:])
                        nc.vector.tensor_add(kr[:], kc_t[:], ks_t[:])
                        return qr, kr, vh

                    pairs = [(n, jp) for n in range(TC)
                             for jp in range(2 * (n + 1))]

                    # ---- Phase 2: attention per head ----
                    with tc.tile_pool(name="p2ps", bufs=2, space="PSUM") as ps2, \
                         tc.tile_pool(name="p2po", bufs=2, space="PSUM") as po2, \
                         tc.tile_pool(name="p2pz", bufs=1, space="PSUM") as pz1:
                        ready = emit_rope(emit_loads(0))
                        nxt = None
                        for h in range(HG):
                            qr, kr, vh = ready
                            chunk_state = {}
                            prev = None
                            fin = None

                            def emit_pv_z(p):
                                n, jp, pexp = p
                                jmax = 4 * (n + 1)
                                ps_o = chunk_state[n][0]
                                for u in range(2):
                                    j = 2 * jp + u
                                    half = slice(u * 512, (u + 1) * 512)
                                    nc.tensor.matmul(
                                        ps_o[:],
                                        vh[:, j * 128:(j + 1) * 128],
                                        pexp[:, half],
                                        start=(j == 0),
                                        stop=(j == jmax - 1),
                                    )
                                if n < PE_Z_CHUNKS:
                                    ps_zz = chunk_state[n][1]
                                    for u in range(2):
                                        j = 2 * jp + u
                                        half = slice(u * 512, (u + 1) * 512)
                                        nc.tensor.matmul(
                                            ps_zz[:],
                                            onescol_sb[:],
                                            pexp[:, half],
                                            start=(j == 0),
                                            stop=(j == jmax - 1),
                                        )

                            def make_finalize(n, h=h):
                                ps_o, zst = chunk_state[n]
                                qsl = slice(n * 512, (n + 1) * 512)

                                def finalize():
                                    rzb = npool.tile(
                                        [128, 512], F32, tag="rzb", name="rzb"
                                    )
                                    if n < PE_Z_CHUNKS:
                                        # zst: ps_zz [1,512] PSUM row
                                        rz = npool.tile(
                                            [1, 512], F32, tag="rz", name="rz"
                                        )
                                        nc.vector.reciprocal(rz[:], zst[:])
                                        nc.gpsimd.partition_broadcast(
                                            rzb[:], rz[:]
                                        )
                                    else:
                                        # zst: zacc bf16 [128,512] SBUF
                                        ps_z = pz1.tile(
                                            [128, 512], F32, tag="pzb", name="ps_z"
                                        )
                                        nc.tensor.matmul(
                                            ps_z[:], ones_sb[:], zst[:],
                                            start=True, stop=True,
                                        )
                                        nc.vector.reciprocal(rzb[:], ps_z[:])
                                    nc.vector.tensor_mul(
                                        attn_sb[h][:, qsl], ps_o[:], rzb[:]
                                    )
                                return finalize

                            for idx, (n, jp) in enumerate(pairs):
                                if jp == 0:
                                    ps_o = po2.tile(
                                        [128, 512], F32, tag="po", name="ps_o"
                                    )
                                    if n < PE_Z_CHUNKS:
                                        zst = pz1.tile(
                                            [1, 512], F32, tag="pzz", name="ps_zz"
                                        )
                                    else:
                                        zst = zpool.tile(
                                            [128, 512], BF16, tag="zacc",
                                            name="zacc",
                                        )
                                    chunk_state[n] = (ps_o, zst)
                                zst = chunk_state[n][1]
                                qsl = slice(n * 512, (n + 1) * 512)
                                ps_s = ps2.tile([128, 1024], F32, tag="ps")
                                for u in range(2):
                                    j = 2 * jp + u
                                    nc.tensor.matmul(
                                        ps_s[:, u * 512:(u + 1) * 512],
                                        kr[:, j * 128:(j + 1) * 128],
                                        qr[:, qsl],
                                        start=True,
                                        stop=True,
                                    )
                                pexp = epool.tile([128, 1024], BF16, tag="pexp")
                                nc.scalar.activation(
                                    pexp[:],
                                    ps_s[:],
                                    mybir.ActivationFunctionType.Exp,
                                    scale=SCALE,
                                )
                                diag = (jp - 2 * n) in (0, 1)
                                if diag:
                                    r2 = jp - 2 * n   # 0 -> r=0,1 ; 1 -> r=2,3
                                    nc.vector.tensor_mul(
                                        pexp[:],
                                        pexp[:],
                                        masks_sb[:, r2 * 1024:(r2 + 1) * 1024],
                                    )
                                if n >= PE_Z_CHUNKS:
                                    # Z partial sums on DVE (bf16, 2x mode);
                                    # <=16 positive terms, ~0.2% rounding
                                    with nc.allow_low_precision(
                                        reason="bf16 softmax-Z partials"
                                    ):
                                        if jp == 0:
                                            nc.vector.tensor_add(
                                                zst[:], pexp[:, 0:512],
                                                pexp[:, 512:1024],
                                            )
                                        else:
                                            nc.vector.tensor_add(
                                                zst[:], zst[:], pexp[:, 0:512]
                                            )
                                            nc.vector.tensor_add(
                                                zst[:], zst[:],
                                                pexp[:, 512:1024],
                                            )
                                if fin is not None and jp == 1:
                                    fin()
                                    fin = None
                                if prev is not None:
                                    emit_pv_z(prev)
                                prev = (n, jp, pexp)
                                if jp == 2 * (n + 1) - 1:
                                    fin = make_finalize(n)
                                # prefetch next head: loads early, rope mid-head
                                if h + 1 < HG:
                                    if idx == 3:
                                        nxt = emit_loads(h + 1)
                                    elif idx == 10:
                                        nxt = emit_rope(nxt)
                            emit_pv_z(prev)
                            fin()
                            if h + 1 < HG:
                                ready = nxt

                    # ---- Phase 3: out projection ----
                    with tc.tile_pool(name="p3w", bufs=2) as wpool3, \
                         tc.tile_pool(name="p3t", bufs=3) as tpool3, \
                         tc.tile_pool(name="p3ps", bufs=2, space="PSUM") as ps3:
                        for n in range(4):
                            woc = wpool3.tile([128, HG * 512], BF16, tag="woc")
                            nc.sync.dma_start(
                                out=woc[:].rearrange("p (h c) -> p h c", h=HG),
                                in_=wo[:, n * 512:(n + 1) * 512].rearrange(
                                    "(h p) c -> p h c", p=128
                                ),
                            )
                            for m in range(TT):
                                ps_y = ps3.tile([128, 512], F32, tag="py")
                                for h in range(HG):
                                    nc.tensor.matmul(
                                        ps_y[:],
                                        attn_sb[h][:, m * 128:(m + 1) * 128],
                                        woc[:, h * 512:(h + 1) * 512],
                                        start=(h == 0),
                                        stop=(h == HG - 1),
                                    )
                                yt = tpool3.tile([128, 512], F32, tag="yt")
                                nc.scalar.copy(yt[:], ps_y[:])
                                nc.sync.dma_start(
                                    out=y[m * 128:(m + 1) * 128,
                                          n * 512:(n + 1) * 512],
                                    in_=yt[:],
                                )

            if iters == 1:
                full_body(None)
            else:
                with tc.For_i(0, iters, 1) as iv:
                    full_body(iv)

    nc.compile()
    return nc


def make_host_inputs(x, Wqkv, bqkv, Wo):
    """Per-core input maps (host-side sharding)."""
    half = D // 2
    freq = np.arange(half, dtype=np.float64)
    theta = 1.0 / (10000.0 ** (2.0 * freq / D))
    pos = np.arange(T, dtype=np.float64)
    ang = pos[:, None] * theta[None, :]          # [T, half]
    sinT = np.sin(ang).T.astype(np.float32)      # [half, T]
    cosT = np.cos(ang).T.astype(np.float32)
    # phase-1 sin table is pre-swapped: after the phase-2 partition swap of
    # the q*sin spill, the effective table is [-sinT; sinT] as RoPE needs
    sin2p = np.concatenate([sinT, -sinT], axis=0).astype(ml_dtypes.bfloat16)
    cos2 = np.concatenate([cosT, cosT], axis=0).astype(ml_dtypes.bfloat16)

    # binary keep masks for the two diagonal pairs of each 512-chunk:
    # pair u covers k-tiles r=2u, 2u+1; keep iff f >= r*128 + p
    masks2 = np.zeros((2, 128, 1024), dtype=np.float32)
    f = np.arange(512)[None, :]
    p = np.arange(128)[:, None]
    for u in range(2):
        for v in range(2):
            r = 2 * u + v
            masks2[u, :, v * 512:(v + 1) * 512] = (f >= r * 128 + p)
    masks2 = masks2.astype(ml_dtypes.bfloat16)
    ones_bf = np.ones((128, 128), dtype=np.float32).astype(ml_dtypes.bfloat16)
    onescol_bf = np.ones((128, 1), dtype=np.float32).astype(ml_dtypes.bfloat16)

    xT = [np.ascontiguousarray(x[b].T) for b in range(B)]
    in_maps = []
    for core in range(N_CORES):
        b, g = core // 2, core % 2
        cs = slice(g * GC, (g + 1) * GC)
        in_maps.append({
            "xT": xT[b],
            "wq": np.ascontiguousarray(Wqkv[:, :C][:, cs]),
            "wk": np.ascontiguousarray(Wqkv[:, C:2 * C][:, cs]),
            "wv": np.ascontiguousarray(Wqkv[:, 2 * C:][:, cs]),
            "bq": np.ascontiguousarray(bqkv[:C][cs].reshape(GC, 1)),
            "bk": np.ascontiguousarray(bqkv[C:2 * C][cs].reshape(GC, 1)),
            "bvb": np.ascontiguousarray(
                np.tile(bqkv[2 * C:][cs].reshape(1, GC), (128, 1))
            ),
            "wo": np.ascontiguousarray(Wo[cs, :]).astype(ml_dtypes.bfloat16),
            "sin2p": sin2p,
            "cos2": cos2,
            "masks2": masks2,
            "ones_bf": ones_bf,
            "onescol_bf": onescol_bf,
        })
    return in_maps


_PROGRAM_CACHE = {}


def get_program(iters=1):
    if iters not in _PROGRAM_CACHE:
        _PROGRAM_CACHE[iters] = build_program(iters)
    return _PROGRAM_CACHE[iters]


def kernel(x, Wqkv, bqkv, Wo, bo):
    x = np.asarray(x, dtype=np.float32)
    Wqkv = np.asarray(Wqkv, dtype=np.float32)
    bqkv = np.asarray(bqkv, dtype=np.float32)
    Wo = np.asarray(Wo, dtype=np.float32)
    bo = np.asarray(bo, dtype=np.float32)

    nc = get_program(1)
    in_maps = make_host_inputs(x, Wqkv, bqkv, Wo)
    res = run_bass_kernel_spmd(nc, in_maps, list(range(N_CORES)))

    out = np.empty((B, T, C), dtype=np.float32)
    for b in range(B):
        out[b] = res.results[2 * b]["y"] + res.results[2 * b + 1]["y"] + bo
    return out
